# revision 49
# baseline (speedup 1.0000x reference)
"""Trainium2 Bass kernel for nn_Decay (gated decay-memory block).

  gate  = sigmoid(x @ Wg + bg)
  store = (x @ Wv) * gate * scale          scale = sqrt(1 - decay)
  mem   = decay-scan(store)                y_t = store_t + decay * y_{t-1}
  que   = sigmoid(x @ Wq + bq)
  out   = (mem * que * scale) @ Wo

Sharding (8 cores): core c handles batch b = c//2, token half h = c%2
(2048 output tokens each).  The decay scan needs history: instead of a
token halo, each quarter's final mem column is AllGather'd between the
(h=0, h=1) pair after its last block, and the h=1 core patches its
first output block's l0 with carry * decay^(t+1) * que (an exact
correction; decay^(t+1) underflows past t~350).  The decay-power vector
is supplied per-core and zeroed on h=0 cores, so the program is uniform
SPMD.

Precision plan (tolerance rel 2e-2; predicted 1.39e-2 on real inputs):
 - V path and O path in bf16 (error contribution ~3e-3)
 - gate/que GEMMs: K rows 0..KF-1 in fp8 e4m3 with DoubleRow perf mode
   (1.44x PE rate), remainder rows in bf16.  Both fp8 operands are
   pre-scaled by 8 host-side (product 64) and the bf16 remainder weights
   by 64, so one PSUM accumulates 64*z; the sigmoid applies scale=1/64.
 - scan state/input fp32; que/l0/weights/x bf16; PSUM fp32; out fp32.

Layout: on-chip [feature (partitions), token (free)].  Free dim 512
(halo block 256) so f32-era LDWEIGHTS leak is amortized; bf16 gets FWL.

Schedule: 4 m-quarter phases x 5 token blocks; que-projection (pq) for
block i runs during block i+1 (and the last block's pq drains into the
next phase / the C transition), so wq loads and phase-boundary weight
loads always have a full block of PE work as cover.  Phase C (output
projection) keeps all four Wo e-quarters resident by reusing SBUF tag
space freed by the A-phase weights.
"""

import sys

sys.path.insert(0, "/opt/trn_rl_repo")

import ml_dtypes
import numpy as np

import concourse.bass as bass
import concourse.tile as tile
from concourse import bacc, mybir
from concourse.bass_utils import run_bass_kernel_spmd

# Problem constants (hardcoded per harness contract)
B, S, E, M = 4, 4096, 2048, 2048
DECAY = 0.95
SCALE = float(np.sqrt(1.0 - DECAY))

N_CORES = 8
OUT_T = S // 2        # 2048 output tokens per core
T = OUT_T             # no halo: cross-core carry instead
P = 128
KF = 1536             # fp8 K-prefix for gate/que projections
KR = E - KF
EC8 = KF // 256       # DoubleRow k-pair count (4)
ECR = KR // 128       # bf16 remainder k-chunks (8)
EC = E // P           # 16
MT = M // P           # 16
MQ = 4                # m-quarter phases
MT_Q = MT // MQ       # 4 m-tiles per quarter
MQW = MT_Q * P        # 512
BLK = [(0, 512), (512, 512), (1024, 512), (1536, 512)]
NB = len(BLK)
GROUPS = [[0, 1], [2, 3], [4, 5], [6, 7]]  # (h=0, h=1) pairs per batch
XS = 8.0              # fp8 per-operand scale (product 64)
WS = XS * XS
F32 = mybir.dt.float32
BF16 = mybir.dt.bfloat16
FP8 = mybir.dt.float8e4
DR = mybir.MatmulPerfMode.DoubleRow
SIG = mybir.ActivationFunctionType.Sigmoid


def build_module(has_bias):
    nc = bacc.Bacc()

    # All tensors are pre-tiled host-side so every DMA is >=4KB contiguous
    # per partition: the DMA engines are descriptor-bound (~42ns/descriptor,
    # one per contiguous run), so [feature, token]-major layouts with 1KB
    # rows throttle transfers to ~24GB/s and stall the issuing engines.
    xT_d = nc.dram_tensor("xT16", [NB, P, EC, 512], BF16, kind="ExternalInput")
    x8_d = nc.dram_tensor("xT8", [NB, P, EC8, 2, 512], FP8, kind="ExternalInput")
    wv_d = nc.dram_tensor("Wv16", [MQ, P, EC, MQW], BF16, kind="ExternalInput")
    wg8_d = nc.dram_tensor("Wg8", [MQ, P, EC8, 2, MQW], FP8, kind="ExternalInput")
    wg16_d = nc.dram_tensor("Wg16", [MQ, P, ECR, MQW], BF16, kind="ExternalInput")
    wq8_d = nc.dram_tensor("Wq8", [MQ, P, EC8, 2, MQW], FP8, kind="ExternalInput")
    wq16_d = nc.dram_tensor("Wq16", [MQ, P, ECR, MQW], BF16, kind="ExternalInput")
    wo_d = nc.dram_tensor("Wo16", [MQ, P, MT, MQW], BF16, kind="ExternalInput")
    dvec_d = nc.dram_tensor("dvec", [P, 512], BF16, kind="ExternalInput")
    if has_bias:
        bg_d = nc.dram_tensor("bg", [M], F32, kind="ExternalInput")
        bq_d = nc.dram_tensor("bq", [M], F32, kind="ExternalInput")
    outT_d = nc.dram_tensor("outT", [NB, P, EC, 512], F32, kind="ExternalOutput")
    l0_d = nc.dram_tensor("l0buf", [NB, P, MT, 512], BF16)  # internal spill
    cin_d = nc.dram_tensor("cin", [MQ, P, MT_Q], F32)  # my carry per quarter
    cout_d = nc.dram_tensor("cout", [2, MQ, P, MT_Q], F32)  # gathered pair
    qsp_d = nc.dram_tensor("qsp", [MQ, P, MT_Q, 512], BF16)  # que(B0) spill

    with tile.TileContext(nc) as tc:
        with (
            tc.tile_pool(name="w", bufs=2) as wp,
            tc.tile_pool(name="a", bufs=2) as sp,
            tc.tile_pool(name="ps", bufs=2, space="PSUM") as ps,
        ):
            consts = sp.tile([P, 512 + 2 * MT], F32, tag="consts", bufs=1)
            nc.vector.memset(consts[:, 0:512], DECAY)
            if has_bias:
                nc.sync.dma_start(
                    out=consts[:, 512 : 512 + MT],
                    in_=bg_d.rearrange("(c p) -> p c", p=P),
                )
                nc.sync.dma_start(
                    out=consts[:, 512 + MT : 512 + 2 * MT],
                    in_=bq_d.rearrange("(c p) -> p c", p=P),
                )
            decay_t = consts[:, 0:512]
            dvec_t = sp.tile([P, 512], BF16, tag="dvec", bufs=1, name="dvec")
            nc.sync.dma_start(out=dvec_t, in_=dvec_d[:, :])

            def bias_ap(kind, mtg):
                if not has_bias:
                    return 0.0
                off = 512 + (0 if kind == "g" else MT) + mtg
                return consts[:, off : off + 1]

            W16 = [P, EC, MQW]  # 16KB/partition: wv / wo / (padded) wg16, wq16

            def load_wv(q):
                t = wp.tile(W16, BF16, tag="wv", name=f"wv{q}")
                nc.scalar.dma_start(out=t, in_=wv_d[q, :, :, :])
                return t

            def load_w8(d, q, tag, nm, eng=None):
                t = wp.tile([P, EC8, 2, MQW], FP8, tag=tag, name=nm)
                (eng or nc.scalar).dma_start(out=t, in_=d[q, :, :, :, :])
                return t

            def load_w16(d, q, tag, nm, eng=None):
                t = wp.tile([P, ECR, MQW], BF16, tag=tag, name=nm)
                (eng or nc.scalar).dma_start(out=t, in_=d[q, :, :, :])
                return t

            def load_wo(eq, tag):
                t = wp.tile(W16, BF16, tag=tag, name=f"wo{eq}")
                nc.scalar.dma_start(out=t, in_=wo_d[eq, :, :, :])
                return t

            def load_x(s):
                # x8 first: the first block's DR matmuls need only x8 + wg8
                q, i = divmod(s, NB)
                x8t = sp.tile(
                    [P, EC8, 2, 512], FP8, tag="x8", bufs=2, name=f"x8_{q}_{i}"
                )
                nc.sync.dma_start(out=x8t, in_=x8_d[i, :, :, :, :])
                xt = sp.tile([P, EC, 512], BF16, tag="xt", bufs=3, name=f"xt{q}_{i}")
                nc.sync.dma_start(out=xt, in_=xT_d[i, :, :, :])
                return xt, x8t

            def emit_pq(inf):
                # deferred que-projection + load for a previous block
                q_, i_ = inf["q"], inf["i"]
                if i_ == 0:
                    # first block's que is re-read by the carry patch: write
                    # the sigmoids into one grouped tile and spill it whole
                    quegrp = sp.tile(
                        [P, MT_Q, 512], BF16, tag="pt", bufs=2, name=f"qg{q_}"
                    )
                l0grp = sp.tile(
                    [P, MT_Q, 512], BF16, tag="l0", bufs=2, name=f"l0_{q_}_{i_}"
                )
                for mt in range(MT_Q):
                    msl = slice(mt * P, (mt + 1) * P)
                    mtg = q_ * MT_Q + mt
                    pqp = ps.tile(
                        [P, 512], F32, tag="pq", bufs=2, name=f"pq{q_}_{i_}_{mt}"
                    )
                    for c2 in range(EC8):
                        nc.tensor.matmul(
                            pqp, lhsT=inf["q8"][:, c2, :, msl],
                            rhs=inf["x8"][:, c2, :, :],
                            start=(c2 == 0), stop=False, perf_mode=DR,
                        )
                    for ec in range(ECR):
                        nc.tensor.matmul(
                            pqp, lhsT=inf["q16"][:, ec, msl],
                            rhs=inf["xt"][:, 2 * EC8 + ec, :],
                            start=False, stop=(ec == ECR - 1),
                        )
                    if i_ == 0:
                        que = quegrp[:, mt, :]
                    else:
                        que = sp.tile(
                            [P, 512], BF16, tag="que", bufs=2,
                            name=f"que{q_}_{i_}_{mt}",
                        )
                    nc.scalar.activation(
                        que, pqp, SIG, bias=bias_ap("q", mtg), scale=1.0 / WS
                    )
                    nc.vector.tensor_mul(
                        l0grp[:, mt, :], inf["mem"][:, mt, :], que
                    )
                if i_ == 0:
                    nc.scalar.dma_start(out=qsp_d[q_, :, :, :], in_=quegrp)
                nc.gpsimd.dma_start(
                    out=l0_d[i_, :, q_ * MT_Q : (q_ + 1) * MT_Q, :], in_=l0grp
                )

            def emit_patch(q):
                # l0(q, B0) += peer_carry[m] * decay^(t+1) * que  (dvec is
                # zero on h=0 cores).  DRAM round-trip on the spilled l0.
                c0 = sp.tile([P, MT_Q], F32, tag="c0", bufs=2, name=f"c0_{q}")
                nc.scalar.dma_start(out=c0, in_=cout_d[0, q, :, :])
                quet = sp.tile(
                    [P, MT_Q, 512], BF16, tag="pt", bufs=2, name=f"quet{q}"
                )
                nc.scalar.dma_start(out=quet, in_=qsp_d[q, :, :, :])
                l0t = sp.tile(
                    [P, MT_Q, 512], BF16, tag="pt", bufs=2, name=f"l0t{q}"
                )
                nc.scalar.dma_start(
                    out=l0t, in_=l0_d[0, :, q * MT_Q : (q + 1) * MT_Q, :]
                )
                for mt in range(MT_Q):
                    cd = sp.tile(
                        [P, 512], F32, tag="store", bufs=2, name=f"cd{q}_{mt}"
                    )
                    nc.vector.tensor_scalar_mul(cd, dvec_t, c0[:, mt : mt + 1])
                    nc.vector.tensor_mul(cd, cd, quet[:, mt, :])
                    nc.vector.tensor_add(l0t[:, mt, :], l0t[:, mt, :], cd)
                nc.scalar.dma_start(
                    out=l0_d[0, :, q * MT_Q : (q + 1) * MT_Q, :], in_=l0t
                )

            # ---- Phase A: 4 m-quarters x 4 token blocks ----
            steps = [(q, i) for q in range(MQ) for i in range(NB)]
            xts = {0: load_x(0)}
            # startup: spread the q0 weight loads over distinct DMA queues so
            # the first blocks' pv/pg aren't serialized behind one queue
            cur = {
                "g8": load_w8(wg8_d, 0, "wg8", "wg8_0"),
                "g16": load_w16(wg16_d, 0, "wg16", "wg16_0"),
                "wv": load_wv(0),
                "q8": load_w8(wq8_d, 0, "wq8", "wq8_0"),
                "q16": load_w16(wq16_d, 0, "wq16", "wq16_0"),
            }
            nxt = {}
            wo_t = {}
            prev = None      # deferred-pq info from previous block
            mem_prev = None  # previous block's mem (scan chain)

            for s, (q, i) in enumerate(steps):
                t0, tsz = BLK[i]
                if i == 0 and q > 0:
                    cur = nxt
                    nxt = {}
                xt, x8t = xts.pop(s)
                if s + 1 < len(steps):
                    xts[s + 1] = load_x(s + 1)

                # phase-boundary weight prefetches (a quarter ahead / wo)
                if q < MQ - 1:
                    if i == 1:
                        nxt["wv"] = load_wv(q + 1)
                    elif i == 2:
                        nxt["g8"] = load_w8(wg8_d, q + 1, "wg8", f"wg8_{q+1}")
                        nxt["g16"] = load_w16(wg16_d, q + 1, "wg16", f"wg16_{q+1}")
                    elif i == 3:
                        nxt["q8"] = load_w8(wq8_d, q + 1, "wq8", f"wq8_{q+1}")
                        nxt["q16"] = load_w16(wq16_d, q + 1, "wq16", f"wq16_{q+1}")
                else:
                    if i == 1:
                        wo_t[2] = load_wo(2, "wo")
                    elif i == 2:
                        wo_t[0] = load_wo(0, "wv")
                    elif i == 3:
                        wo_t[3] = load_wo(3, "wo")

                # deferred pq for the previous block (keeps PE busy while
                # this block's x/weights stream in)
                if prev is not None:
                    emit_pq(prev)



                # pg first (DR matmuls need only x8 + wg8 — shortest startup
                # dependency), then pv; gates are ready by the time the
                # store-muls consume the pv psums, so pv's ring never cycles
                # into an unmet DVE dependency.
                mem_t = sp.tile(
                    [P, MT_Q, 512], F32, tag="mem", bufs=2, name=f"mem{q}_{i}"
                )
                gates = []
                for mt in range(MT_Q):
                    msl = slice(mt * P, (mt + 1) * P)
                    mtg = q * MT_Q + mt
                    pgp = ps.tile(
                        [P, 512], F32, tag="pg", bufs=2, name=f"pg{q}_{i}_{mt}"
                    )[:, :tsz]
                    for c2 in range(EC8):
                        nc.tensor.matmul(
                            pgp, lhsT=cur["g8"][:, c2, :, msl],
                            rhs=x8t[:, c2, :, :tsz],
                            start=(c2 == 0), stop=False, perf_mode=DR,
                        )
                    for ec in range(ECR):
                        nc.tensor.matmul(
                            pgp, lhsT=cur["g16"][:, ec, msl],
                            rhs=xt[:, 2 * EC8 + ec, :tsz],
                            start=False, stop=(ec == ECR - 1),
                        )
                    gate = sp.tile(
                        [P, 512], BF16, tag="gate", bufs=2, name=f"gate{q}_{i}_{mt}"
                    )[:, :tsz]
                    nc.scalar.activation(
                        gate, pgp, SIG, bias=bias_ap("g", mtg), scale=1.0 / WS
                    )
                    gates.append(gate)
                for mt in range(MT_Q):
                    msl = slice(mt * P, (mt + 1) * P)
                    pvp = ps.tile(
                        [P, 512], F32, tag="pv", bufs=3, name=f"pv{q}_{i}_{mt}"
                    )[:, :tsz]
                    for ec in range(EC):
                        nc.tensor.matmul(
                            pvp, lhsT=cur["wv"][:, ec, msl], rhs=xt[:, ec, :tsz],
                            start=(ec == 0), stop=(ec == EC - 1),
                        )
                    store = sp.tile(
                        [P, 512], F32, tag="store", bufs=2, name=f"st{q}_{i}_{mt}"
                    )[:, :tsz]
                    nc.vector.tensor_mul(store, pvp, gates[mt])
                    init = (
                        0.0
                        if i == 0
                        else mem_prev[:, mt, BLK[i - 1][1] - 1 : BLK[i - 1][1]]
                    )
                    nc.vector.tensor_tensor_scan(
                        mem_t[:, mt, :tsz], decay_t[:, :tsz], store,
                        initial=init,
                        op0=mybir.AluOpType.mult, op1=mybir.AluOpType.add,
                    )

                if i == NB - 1:
                    # stage this quarter's scan carry; one AllGather at the
                    # drain exchanges all four at once (collectives block the
                    # gpsimd queue, so mid-stream ones backpressure spills)
                    nc.gpsimd.dma_start(
                        out=cin_d[q, :, :], in_=mem_t[:, :, 511]
                    )

                prev = dict(
                    q=q, i=i, mem=mem_t, xt=xt, x8=x8t,
                    q8=cur["q8"], q16=cur["q16"],
                )
                mem_prev = mem_t

            emit_pq(prev)  # drain: pq for (3, B3) covers the C transition
            wo_t[1] = load_wo(1, "wv")

            # ---- Phase C: output projection, all Wo quarters resident ----
            # token block 0 is processed LAST: its l0 is rewritten by the
            # carry patches (issued below), and q3's carry only lands after
            # the drain.  The first two lt loads precede the collective in
            # the queues so C starts immediately.
            C_ORDER = [1, 2, 3, 0]
            lts = {}

            def load_lt(ci):
                nb_ = C_ORDER[ci]
                t = sp.tile([P, MT, 512], BF16, tag="xt", bufs=3, name=f"lt_{ci}")
                nc.sync.dma_start(out=t, in_=l0_d[nb_, :, :, :])
                lts[ci] = t

            load_lt(0)
            load_lt(1)
            nc.gpsimd.collective_compute(
                "AllGather", mybir.AluOpType.bypass,
                replica_groups=GROUPS,
                ins=[cin_d[:, :, :]], outs=[cout_d[:, :, :, :]],
            )

            for ci, tb in enumerate(C_ORDER):
                if ci == 1:
                    # patches run here — after block ci=0's DVE work, so the
                    # in-order DVE FIFO never parks on the AllGather, and
                    # before lt(0) (tb=0, loaded at ci=1) reads patched l0
                    for pq_ in range(MQ):
                        emit_patch(pq_)
                if ci + 2 < len(C_ORDER):
                    load_lt(ci + 2)
                lt = lts.pop(ci)
                for eq in range(4):
                    ot = sp.tile(
                        [P, MT_Q, 512], F32, tag="mem", bufs=2, name=f"ot{eq}_{tb}"
                    )
                    for et in range(MT_Q):
                        pop = ps.tile(
                            [P, 512], F32, tag="pv", bufs=3, name=f"po{eq}_{tb}_{et}"
                        )
                        for mc in range(MT):
                            nc.tensor.matmul(
                                pop,
                                lhsT=wo_t[eq][:, mc, et * P : (et + 1) * P],
                                rhs=lt[:, mc, :],
                                start=(mc == 0), stop=(mc == MT - 1),
                            )
                        nc.vector.tensor_copy(ot[:, et, :], pop)
                    nc.sync.dma_start(
                        out=outT_d[tb, :, eq * MT_Q : (eq + 1) * MT_Q, :], in_=ot
                    )
    nc.compile()
    return nc


_cached = {}


def _get_module(has_bias):
    if has_bias not in _cached:
        _cached[has_bias] = build_module(has_bias)
    return _cached[has_bias]


def _q8(a):
    return np.clip(a * np.float32(XS), -240, 240).astype(ml_dtypes.float8_e4m3)


def _tile_k(w):
    # [K=2048 rows, 512 cols] -> [512-col quarters, P, k-chunk, 512]
    kc = w.shape[0] // P
    nq = w.shape[1] // MQW
    return np.ascontiguousarray(
        w.reshape(kc, P, nq, MQW).transpose(2, 1, 0, 3)
    )


def _tile_k8(w):
    # fp8 DoubleRow pairs: [KF rows, cols] -> [quarters, P, c2, 2, cols]
    nq = w.shape[1] // MQW
    return np.ascontiguousarray(
        w.reshape(EC8, 2, P, nq, MQW).transpose(3, 2, 0, 1, 4)
    )


def _prep_inputs(x, Wv, Wg, bg, Wq, bq, Wo, has_bias):
    """Shard + quantize + pre-tile host-side. Returns per-core input dicts."""
    bf = ml_dtypes.bfloat16
    x = np.asarray(x, dtype=np.float32)
    Wv16 = _tile_k((np.asarray(Wv, np.float32) * np.float32(SCALE)).astype(bf))
    Wo16 = _tile_k((np.asarray(Wo, np.float32) * np.float32(SCALE)).astype(bf))
    Wg = np.asarray(Wg, np.float32)
    Wq = np.asarray(Wq, np.float32)
    Wg8, Wq8 = _tile_k8(_q8(Wg[:KF])), _tile_k8(_q8(Wq[:KF]))
    Wg16 = _tile_k((Wg[KF:] * np.float32(WS)).astype(bf))
    Wq16 = _tile_k((Wq[KF:] * np.float32(WS)).astype(bf))
    dvec = np.tile(
        np.float32(DECAY) ** np.arange(1, 513, dtype=np.float32), (P, 1)
    ).astype(bf)
    zvec = np.zeros((P, 512), dtype=bf)
    in_maps = []
    for c in range(N_CORES):
        b, h = c // 2, c % 2
        xTc = np.ascontiguousarray(x[b, h * OUT_T : (h + 1) * OUT_T].T)
        x16 = xTc.astype(bf)
        m = {
            # [E, T] -> [block, P, e-chunk, 512]
            "xT16": np.ascontiguousarray(
                x16.reshape(EC, P, NB, 512).transpose(2, 1, 0, 3)
            ),
            "xT8": np.ascontiguousarray(
                _q8(xTc[:KF]).reshape(EC8, 2, P, NB, 512).transpose(3, 2, 0, 1, 4)
            ),
            "Wv16": Wv16, "Wg8": Wg8, "Wg16": Wg16,
            "Wq8": Wq8, "Wq16": Wq16, "Wo16": Wo16,
            "dvec": dvec if h == 1 else zvec,
        }
        if has_bias:
            m["bg"] = np.ascontiguousarray(bg, dtype=np.float32)
            m["bq"] = np.ascontiguousarray(bq, dtype=np.float32)
        in_maps.append(m)
    return in_maps


def run(x, Wv, Wg, bg, Wq, bq, Wo, trace=False):
    bg = np.asarray(bg, dtype=np.float32)
    bq = np.asarray(bq, dtype=np.float32)
    has_bias = bool(np.any(bg)) or bool(np.any(bq))
    nc = _get_module(has_bias)
    in_maps = _prep_inputs(x, Wv, Wg, bg, Wq, bq, Wo, has_bias)
    res = run_bass_kernel_spmd(
        nc, in_maps, core_ids=list(range(N_CORES)), trace=trace
    )
    out = np.empty((B, S, E), dtype=np.float32)
    for c in range(N_CORES):
        b, h = c // 2, c % 2
        # [block, P, e-chunk, 512] -> [token, e]
        oc = res.results[c]["outT"].transpose(0, 3, 2, 1).reshape(OUT_T, E)
        out[b, h * OUT_T : (h + 1) * OUT_T] = oc
    return out, res


def kernel(**inputs):
    out, _ = run(**inputs)
    return out
